# revision 10
# baseline (speedup 1.0000x reference)
"""Trainium2 Bass kernel for AetherLoss: chamfer(recon_x, x) + beta*KL(mu, logvar).

Strategy: data-parallel over batch B=8 across 8 NeuronCores (1 point-cloud
pair + 1 latent row per core).  Per core, the 4096x4096 *negated* squared
distance matrix  -dist[n,m] = 2*x_n.y_m - |x_n|^2 - |y_m|^2  is produced by
the TensorEngine as a single K=24 matmul per tile via augmented vectors,
where every fp32 operand is split into 3 bf16 components (hi/mid/lo) so the
bf16 PE path reproduces fp32-accurate products (err ~1e-7 relative).
Row mins (min over y for each x) come from DVE free-axis max-reduce of the
negated tiles; col mins from a DVE running elementwise max plus a GPSIMD
partition all-reduce tail.  Per-core partial sums are combined on the host
(equal shard sizes -> plain means), which is the scalar "all-reduce".
"""

import numpy as np
from contextlib import ExitStack

B, D, N = 8, 3, 4096
LATENT = 256
NCORES = 8
BETA = 1.0

PT = 128            # x-tile size (matmul output partitions)
NT = N // PT        # 32 x-tiles
FC = 2048           # psum group free size (4 banks)
NG = N // FC        # 2 groups
CH = 512            # matmul moving free dim (1 psum bank)
CPG = FC // CH      # 4 chunks per group
K = 24              # augmented contraction size

_cache = {}


def _build_program():
    import concourse.bass as bass
    import concourse.tile as tile
    from concourse import bacc, mybir, bass_isa

    f32 = mybir.dt.float32
    bf16 = mybir.dt.bfloat16

    nc = bacc.Bacc(trn_type="TRN2", debug=False, target_bir_lowering=False)

    # ---- per-core DRAM I/O (SPMD: same program, per-core data) ----
    xr = nc.dram_tensor("xr", [D, N], f32, kind="ExternalInput")      # recon_x[b]
    xx = nc.dram_tensor("xx", [D, N], f32, kind="ExternalInput")      # x[b]
    mu = nc.dram_tensor("mu", [LATENT], f32, kind="ExternalInput")
    lv = nc.dram_tensor("lv", [LATENT], f32, kind="ExternalInput")

    o_row = nc.dram_tensor("o_row", [128, NT * NG], f32, kind="ExternalOutput")
    o_col = nc.dram_tensor("o_col", [N], f32, kind="ExternalOutput")
    o_kl = nc.dram_tensor("o_kl", [128, 1], f32, kind="ExternalOutput")

    # internal DRAM staging for the [96,128] -> [3,4096] layout flatten
    st = {}
    for name in ("axh", "axm", "axl", "x2", "yh", "ym", "yl", "y2"):
        st[name] = nc.dram_tensor("st_" + name, [D * N], bf16)

    with tile.TileContext(nc) as tc, ExitStack() as ctx:
        const = ctx.enter_context(tc.tile_pool(name="const", bufs=1))
        work = ctx.enter_context(tc.tile_pool(name="work", bufs=1))
        psum = ctx.enter_context(tc.tile_pool(name="psum", bufs=2, space="PSUM"))

        # ================= KL term (tiny; schedule early) =================
        mu2d = work.tile([128, LATENT // 128], f32, tag="mu2d")
        lv2d = work.tile([128, LATENT // 128], f32, tag="lv2d")
        nc.sync.dma_start(mu2d[:], mu.ap().rearrange("(p f) -> p f", p=128))
        nc.sync.dma_start(lv2d[:], lv.ap().rearrange("(p f) -> p f", p=128))
        klsq = work.tile([128, LATENT // 128], f32, tag="klsq")
        klex = work.tile([128, LATENT // 128], f32, tag="klex")
        klt = work.tile([128, LATENT // 128], f32, tag="klt")
        klp = work.tile([128, 1], f32, tag="klp")
        nc.vector.tensor_tensor(klsq[:], mu2d[:], mu2d[:], op=mybir.AluOpType.mult)
        nc.scalar.activation(klex[:], lv2d[:], mybir.ActivationFunctionType.Exp)
        nc.vector.tensor_tensor(klt[:], lv2d[:], klsq[:], op=mybir.AluOpType.subtract)
        nc.vector.tensor_tensor(klt[:], klt[:], klex[:], op=mybir.AluOpType.subtract)
        nc.vector.reduce_sum(klp[:], klt[:], axis=mybir.AxisListType.X)
        nc.sync.dma_start(o_kl.ap(), klp[:])

        # ================= aug operand prep =================
        # Load [3,4096] as [96,128]: partition p = d*32 + t, free n (128).
        def load96(dram):
            t = work.tile([96, 128], f32, tag=f"ld_{dram.name}", name=f"ld_{dram.name}")
            nc.sync.dma_start(t[:], dram.ap().rearrange("d (t n) -> (d t) n", n=128))
            return t

        def split3(src_f32, base, scale=None):
            """3-way bf16 split of an fp32 tile; returns (h, m, l) bf16 tiles."""
            p, fd = src_f32.shape
            h = work.tile([p, fd], bf16, tag=f"{base}_h", name=f"{base}_h")
            m = work.tile([p, fd], bf16, tag=f"{base}_m", name=f"{base}_m")
            l = work.tile([p, fd], bf16, tag=f"{base}_l", name=f"{base}_l")
            r = work.tile([p, fd], f32, tag=f"{base}_r", name=f"{base}_r")
            r2 = work.tile([p, fd], f32, tag=f"{base}_r2", name=f"{base}_r2")
            nc.vector.tensor_copy(h[:], src_f32[:])
            nc.vector.tensor_tensor(r[:], src_f32[:], h[:], op=mybir.AluOpType.subtract)
            nc.vector.tensor_copy(m[:], r[:])
            nc.vector.tensor_tensor(r2[:], r[:], m[:], op=mybir.AluOpType.subtract)
            nc.vector.tensor_copy(l[:], r2[:])
            return h, m, l

        def neg_sumsq(dram, base):
            """-sum_d x[d,n]^2 as a [128, 32] fp32 tile (partition=n%128, free=tile).

            Loads x in point-major layout [128, (t, d)] so the d-sum is a
            free-axis segmented reduce (engines cannot cross partitions).
            """
            xt = work.tile([128, 96], f32, tag=f"{base}_xt", name=f"{base}_xt")
            nc.sync.dma_start(
                xt[:], dram.ap().rearrange("d (t p) -> p d t", p=128))
            sq = work.tile([128, 96], f32, tag=f"{base}_sqt", name=f"{base}_sqt")
            nc.vector.tensor_tensor(sq[:], xt[:], xt[:], op=mybir.AluOpType.mult)
            s = work.tile([128, 32], f32, tag=f"{base}_s", name=f"{base}_s")
            nc.vector.tensor_reduce(
                s[:], sq[:].rearrange("p (d t) -> p t d", t=32),
                axis=mybir.AxisListType.X, op=mybir.AluOpType.add)
            out = work.tile([128, 32], f32, tag=f"{base}_ss", name=f"{base}_ss")
            nc.vector.tensor_scalar_mul(out[:], s[:], -1.0)
            return out

        # x side (stationary / lhsT), carries the +2 scale and the negations
        x96 = load96(xr)
        ax = work.tile([96, 128], f32, tag="ax")
        nc.vector.tensor_scalar_mul(ax[:], x96[:], 2.0)
        axh, axm, axl = split3(ax, "ax")
        nx2 = neg_sumsq(xr, "x2")              # -|x|^2, [128, 32]
        x2h, x2m, x2l = split3(nx2, "x2")

        # y side (moving / rhs)
        y96 = load96(xx)
        yh, ym, yl = split3(y96, "y")
        ny2 = neg_sumsq(xx, "y2")              # -|y|^2, [128, 32]
        y2h, y2m, y2l = split3(ny2, "y2")

        # stage components to DRAM, flattened to the [3, 4096] row layout
        def stage96(name, t):
            # [96, 128] tile, partition p = d*32+t -> dram offset p*128+n
            nc.sync.dma_start(
                st[name].ap().rearrange("(p n) -> p n", n=128), t[:])

        def stage_trio_T(name, parts):
            # [128, 32] tiles (partition=point%128, free=tile) -> rows of [3,4096]
            for i, p in enumerate(parts):
                nc.sync.dma_start(
                    st[name].ap()[i * N:(i + 1) * N].rearrange(
                        "(t p) -> p t", p=128),
                    p[:],
                )

        stage96("axh", axh); stage96("axm", axm); stage96("axl", axl)
        stage_trio_T("x2", [x2h, x2m, x2l])
        stage96("yh", yh); stage96("ym", ym); stage96("yl", yl)
        stage_trio_T("y2", [y2h, y2m, y2l])

        # assemble augmented operands [24, 4096] bf16
        augX = const.tile([K, N], bf16, tag="augX")
        augY = const.tile([K, N], bf16, tag="augY")

        def fill(dst, rows, src_name):
            nc.sync.dma_start(
                dst[rows:rows + 3, :],
                st[src_name].ap().rearrange("(d m) -> d m", d=3),
            )

        # row pairing: (axh,yh) (axh,ym) (axm,yh) (axh,yl) (axl,yh) (axm,ym)
        #              (x2trio, ones) (ones, y2trio)
        for r, n_ in ((0, "axh"), (3, "axh"), (6, "axm"), (9, "axh"),
                      (12, "axl"), (15, "axm"), (18, "x2")):
            fill(augX, r, n_)
        for r, n_ in ((0, "yh"), (3, "ym"), (6, "yh"), (9, "yl"),
                      (12, "yh"), (15, "ym"), (21, "y2")):
            fill(augY, r, n_)
        ones3 = work.tile([3, N], bf16, tag="ones3")
        nc.vector.memset(ones3[:], 1.0)
        nc.sync.dma_start(augX[21:24, :], ones3[:])
        nc.sync.dma_start(augY[18:21, :], ones3[:])

        # ================= main loop =================
        rowmax = const.tile([128, NT * NG], f32, tag="rowmax")
        colacc = [const.tile([128, FC], f32, tag=f"colacc{g}", name=f"colacc{g}") for g in range(NG)]

        for pt in range(NT):
            lhsT = augX[:, pt * PT:(pt + 1) * PT]
            for g in range(NG):
                ptile = psum.tile([128, FC], f32, tag="ptile")
                for q in range(CPG):
                    c = g * CPG + q
                    nc.tensor.matmul(
                        ptile[:, q * CH:(q + 1) * CH],
                        lhsT,
                        augY[:, c * CH:(c + 1) * CH],
                        start=True, stop=True,
                    )
                nc.vector.tensor_reduce(
                    rowmax[:, pt * NG + g: pt * NG + g + 1], ptile[:],
                    axis=mybir.AxisListType.X, op=mybir.AluOpType.max,
                )
                if pt == 0:
                    nc.vector.tensor_copy(colacc[g][:], ptile[:])
                else:
                    nc.vector.tensor_tensor(
                        colacc[g][:], colacc[g][:], ptile[:],
                        op=mybir.AluOpType.max,
                    )

        # ================= tails =================
        nc.sync.dma_start(o_row.ap(), rowmax[:])
        for g in range(NG):
            colred = work.tile([128, FC], f32, tag=f"colred{g}", name=f"colred{g}")
            nc.gpsimd.partition_all_reduce(
                colred[:], colacc[g][:], 128, bass_isa.ReduceOp.max)
            nc.sync.dma_start(o_col.ap()[g * FC:(g + 1) * FC], colred[0:1, :])

    nc.compile()
    return nc


def _get_nc():
    if "nc" not in _cache:
        _cache["nc"] = _build_program()
    return _cache["nc"]


def _register_ntff_hook():
    """This image's antenv lacks axon_hooks; register the NTFF profile hook
    ourselves so run_bass_kernel_spmd(trace=True) can neuron-profile."""
    import sys, types
    if "antenv.axon_hooks" in sys.modules:
        return
    try:
        from trn_agent_boot.trn_boot import _ntff_profile_via_ctypes
        hook = _ntff_profile_via_ctypes("/opt/axon/libaxon_pjrt.so")
        mod = types.ModuleType("antenv.axon_hooks")
        mod.get_axon_ntff_profile_hook = lambda: hook
        mod.set_axon_ntff_profile_hook = lambda h: None
        sys.modules["antenv.axon_hooks"] = mod
        # artifact upload needs a share bucket we don't have; stub it
        from concourse import bass_utils
        bass_utils.upload_artifacts = lambda tmpdir: tmpdir
    except Exception:
        pass


def _run(in_maps, trace=False):
    from concourse.bass_utils import run_bass_kernel_spmd
    if trace:
        _register_ntff_hook()
    nc = _get_nc()
    return run_bass_kernel_spmd(nc, in_maps, list(range(NCORES)), trace=trace)


def _combine(results):
    minx_sum = 0.0
    miny_sum = 0.0
    kl_sum = 0.0
    for r in results:
        rm = r["o_row"].reshape(128, NT, NG).max(axis=2)   # [128, 32] of max(-dist)
        minx_sum += -(rm.astype(np.float64).sum())
        miny_sum += -(r["o_col"].astype(np.float64).sum())
        kl_sum += r["o_kl"].astype(np.float64).sum()
    recon = minx_sum / (NCORES * N) + miny_sum / (NCORES * N)
    kld = -0.5 * (B * LATENT * 1.0 + kl_sum) / B
    total = recon + BETA * kld
    return (np.float32(total), np.float32(recon), np.float32(kld))


def kernel(recon_x, x, mu, logvar, _trace=False):
    recon_x = np.ascontiguousarray(recon_x, dtype=np.float32)
    x = np.ascontiguousarray(x, dtype=np.float32)
    mu = np.ascontiguousarray(mu, dtype=np.float32)
    logvar = np.ascontiguousarray(logvar, dtype=np.float32)
    in_maps = [
        {"xr": recon_x[c], "xx": x[c], "mu": mu[c], "lv": logvar[c]}
        for c in range(NCORES)
    ]
    res = _run(in_maps, trace=_trace)
    out = _combine(res.results)
    if _trace:
        return out, res
    return out


# revision 14
# speedup vs baseline: 367.3450x; 367.3450x over previous
"""Trainium2 Bass kernel for AetherLoss: chamfer(recon_x, x) + beta*KL(mu, logvar).

Strategy: data-parallel over batch B=8 across 8 NeuronCores (1 point-cloud
pair + 1 latent row per core).  Per core, the 4096x4096 *negated* squared
distance matrix  -dist[n,m] = 2*x_n.y_m - |x_n|^2 - |y_m|^2  is produced by
the TensorEngine as a single K=24 matmul per tile via augmented vectors,
where every fp32 operand is split into 3 bf16 components (hi/mid/lo) so the
bf16 PE path reproduces fp32-accurate products (err ~1e-7 relative).
ScalarE stages each PSUM tile to SBUF as fp16, which lets VectorE run both
min-reductions (row max-tree over the free axis + running elementwise max
for columns) in its 2x packed mode.  A GPSIMD partition all-reduce finishes
the column mins.  Per-core partial sums are combined on the host (equal
shard sizes -> plain means), which is the scalar "all-reduce".
"""

import numpy as np
from contextlib import ExitStack

B, D, N = 8, 3, 4096
LATENT = 256
NCORES = 8
BETA = 1.0

PT = 128            # x-tile size (matmul output partitions)
NT = N // PT        # 32 x-tiles
FC = 2048           # psum group free size (4 banks)
NG = N // FC        # 2 groups
CH = 512            # matmul moving free dim (1 psum bank)
CPG = FC // CH      # 4 chunks per group
K = 24              # augmented contraction size

_cache = {}


def _build_program():
    import concourse.bass as bass
    import concourse.tile as tile
    from concourse import bacc, mybir, bass_isa

    f32 = mybir.dt.float32
    f16 = mybir.dt.float16
    bf16 = mybir.dt.bfloat16
    i32 = mybir.dt.int32
    MAX = mybir.AluOpType.max

    nc = bacc.Bacc(trn_type="TRN2", debug=False, target_bir_lowering=False)

    # ---- per-core DRAM I/O (SPMD: same program, per-core data) ----
    xr = nc.dram_tensor("xr", [D, N], f32, kind="ExternalInput")      # recon_x[b]
    xx = nc.dram_tensor("xx", [D, N], f32, kind="ExternalInput")      # x[b]
    mu = nc.dram_tensor("mu", [LATENT], f32, kind="ExternalInput")
    lv = nc.dram_tensor("lv", [LATENT], f32, kind="ExternalInput")

    o_row = nc.dram_tensor("o_row", [128, NT], f32, kind="ExternalOutput")
    o_col = nc.dram_tensor("o_col", [N], f32, kind="ExternalOutput")
    o_kl = nc.dram_tensor("o_kl", [128, 1], f32, kind="ExternalOutput")

    # internal DRAM staging for the [96,128] -> [3,4096] layout flatten
    st = {}
    for name in ("axh", "axm", "axl", "x2", "yh", "ym", "yl", "y2"):
        st[name] = nc.dram_tensor("st_" + name, [D * N], bf16)

    with tile.TileContext(nc) as tc, ExitStack() as ctx:
        const = ctx.enter_context(tc.tile_pool(name="const", bufs=1))
        work = ctx.enter_context(tc.tile_pool(name="work", bufs=1))
        stg = ctx.enter_context(tc.tile_pool(name="stg", bufs=2))
        psum = ctx.enter_context(tc.tile_pool(name="psum", bufs=2, space="PSUM"))

        # ================= KL term (tiny; schedule early) =================
        mu2d = work.tile([128, LATENT // 128], f32, tag="mu2d")
        lv2d = work.tile([128, LATENT // 128], f32, tag="lv2d")
        nc.sync.dma_start(mu2d[:], mu.ap().rearrange("(p f) -> p f", p=128))
        nc.sync.dma_start(lv2d[:], lv.ap().rearrange("(p f) -> p f", p=128))
        klsq = work.tile([128, LATENT // 128], f32, tag="klsq")
        klex = work.tile([128, LATENT // 128], f32, tag="klex")
        klt = work.tile([128, LATENT // 128], f32, tag="klt")
        klp = work.tile([128, 1], f32, tag="klp")
        nc.vector.tensor_tensor(klsq[:], mu2d[:], mu2d[:], op=mybir.AluOpType.mult)
        nc.scalar.activation(klex[:], lv2d[:], mybir.ActivationFunctionType.Exp)
        nc.vector.tensor_tensor(klt[:], lv2d[:], klsq[:], op=mybir.AluOpType.subtract)
        nc.vector.tensor_tensor(klt[:], klt[:], klex[:], op=mybir.AluOpType.subtract)
        nc.vector.reduce_sum(klp[:], klt[:], axis=mybir.AxisListType.X)
        nc.sync.dma_start(o_kl.ap(), klp[:])

        # ================= aug operand prep =================
        # indicator[k, m] = (k % 32 == m), for the PE partition-group sum
        ind_col = work.tile([96, 32], i32, tag="ind_col")
        ind_row = work.tile([96, 1], i32, tag="ind_row")
        ind_mod = work.tile([96, 1], f32, tag="ind_mod")
        ind = work.tile([96, 32], f32, tag="ind")
        for j in range(3):
            # value m + 32*j on partition slice j -> equality below tests k%32==m
            nc.gpsimd.iota(ind_col[32 * j:32 * (j + 1), :], pattern=[[1, 32]],
                           base=32 * j, channel_multiplier=0)
        nc.gpsimd.iota(ind_row[:], pattern=[[0, 1]], base=0, channel_multiplier=1)
        nc.vector.tensor_copy(ind_mod[:], ind_row[:])
        nc.vector.tensor_scalar(ind[:], ind_col[:], ind_mod[:, 0:1], None,
                                op0=mybir.AluOpType.is_equal)

        # Load [3,4096] as [96,128]: partition p = d*32 + t, free n (128).
        def load96(dram):
            t = work.tile([96, 128], f32, tag=f"ld_{dram.name}", name=f"ld_{dram.name}")
            nc.sync.dma_start(t[:], dram.ap().rearrange("d (t n) -> (d t) n", n=128))
            return t

        def split3(src_f32, base):
            """3-way bf16 split of an fp32 tile; returns (h, m, l) bf16 tiles."""
            p, fd = src_f32.shape
            h = work.tile([p, fd], bf16, tag=f"{base}_h", name=f"{base}_h")
            m = work.tile([p, fd], bf16, tag=f"{base}_m", name=f"{base}_m")
            l = work.tile([p, fd], bf16, tag=f"{base}_l", name=f"{base}_l")
            r = work.tile([p, fd], f32, tag=f"{base}_r", name=f"{base}_r")
            r2 = work.tile([p, fd], f32, tag=f"{base}_r2", name=f"{base}_r2")
            nc.vector.tensor_copy(h[:], src_f32[:])
            nc.vector.tensor_tensor(r[:], src_f32[:], h[:], op=mybir.AluOpType.subtract)
            nc.vector.tensor_copy(m[:], r[:])
            nc.vector.tensor_tensor(r2[:], r[:], m[:], op=mybir.AluOpType.subtract)
            nc.vector.tensor_copy(l[:], r2[:])
            return h, m, l

        def neg_sumsq(src96, base):
            """-sum_d src[d*32+t, n]^2 as a [32, 128] fp32 tile via the PE:
            out[t, n] = sum_k ind[k, t] * sq[k, n]."""
            sq = work.tile([96, 128], f32, tag=f"{base}_sq", name=f"{base}_sq")
            nc.vector.tensor_tensor(sq[:], src96[:], src96[:], op=mybir.AluOpType.mult)
            ps = psum.tile([32, 128], f32, tag="ptile", name=f"{base}_ps")
            nc.tensor.matmul(ps[:], ind[:], sq[:], start=True, stop=True)
            out = work.tile([32, 128], f32, tag=f"{base}_ss", name=f"{base}_ss")
            nc.vector.tensor_scalar_mul(out[:], ps[:], -1.0)
            return out

        # x side (stationary / lhsT) carries the +2 scale
        x96 = load96(xr)
        ax = work.tile([96, 128], f32, tag="ax")
        nc.vector.tensor_scalar_mul(ax[:], x96[:], 2.0)
        axh, axm, axl = split3(ax, "ax")
        nx2 = neg_sumsq(x96, "x2")             # -|x|^2, [32, 128]
        x2h, x2m, x2l = split3(nx2, "x2")

        # y side (moving / rhs)
        y96 = load96(xx)
        yh, ym, yl = split3(y96, "y")
        ny2 = neg_sumsq(y96, "y2")             # -|y|^2, [32, 128]
        y2h, y2m, y2l = split3(ny2, "y2")

        # stage components to DRAM, flattened to the [3, 4096] row layout
        def stage(name, t, off=0):
            # [P, 128] tile, partition p -> dram offset (off + p)*128 + n
            rows = t.shape[0]
            nc.sync.dma_start(
                st[name].ap()[off * 128:(off + rows) * 128].rearrange(
                    "(p n) -> p n", n=128),
                t[:])

        stage("axh", axh); stage("axm", axm); stage("axl", axl)
        stage("x2", x2h, 0); stage("x2", x2m, 32); stage("x2", x2l, 64)
        stage("yh", yh); stage("ym", ym); stage("yl", yl)
        stage("y2", y2h, 0); stage("y2", y2m, 32); stage("y2", y2l, 64)

        # assemble augmented operands [24, 4096] bf16
        augX = const.tile([K, N], bf16, tag="augX")
        augY = const.tile([K, N], bf16, tag="augY")

        def fill(dst, rows, src_name):
            nc.sync.dma_start(
                dst[rows:rows + 3, :],
                st[src_name].ap().rearrange("(d m) -> d m", d=3),
            )

        # row pairing: (axh,yh) (axh,ym) (axm,yh) (axh,yl) (axl,yh) (axm,ym)
        #              (x2trio, ones) (ones, y2trio)
        for r, n_ in ((0, "axh"), (3, "axh"), (6, "axm"), (9, "axh"),
                      (12, "axl"), (15, "axm"), (18, "x2")):
            fill(augX, r, n_)
        for r, n_ in ((0, "yh"), (3, "ym"), (6, "yh"), (9, "yl"),
                      (12, "yh"), (15, "ym"), (21, "y2")):
            fill(augY, r, n_)
        ones3 = work.tile([3, N], bf16, tag="ones3")
        nc.vector.memset(ones3[:], 1.0)
        nc.sync.dma_start(augX[21:24, :], ones3[:])
        nc.sync.dma_start(augY[18:21, :], ones3[:])

        # ================= main loop =================
        rowmax = const.tile([128, NT], f32, tag="rowmax")
        colacc = [const.tile([128, FC], f16, tag=f"colacc{g}", name=f"colacc{g}")
                  for g in range(NG)]

        for pt in range(NT):
            lhsT = augX[:, pt * PT:(pt + 1) * PT]
            rowbuf = stg.tile([128, N], f16, tag="rowbuf", name="rowbuf")
            for g in range(NG):
                ptile = psum.tile([128, FC], f32, tag="ptile", name="ptile")
                for q in range(CPG):
                    c = g * CPG + q
                    nc.tensor.matmul(
                        ptile[:, q * CH:(q + 1) * CH],
                        lhsT,
                        augY[:, c * CH:(c + 1) * CH],
                        start=True, stop=True,
                    )
                # ScalarE stages fp32 PSUM -> fp16 SBUF
                rslice = rowbuf[:, g * FC:(g + 1) * FC]
                nc.scalar.copy(rslice, ptile[:])
                if pt == 0:
                    nc.vector.tensor_copy(colacc[g][:], rslice)
                else:
                    nc.vector.tensor_tensor(colacc[g][:], colacc[g][:], rslice,
                                            op=MAX)
            # fp16 max-tree over the free axis (2x packed mode), then reduce
            t1 = stg.tile([128, 2048], f16, tag="t1", name="t1")
            nc.vector.tensor_tensor(t1[:], rowbuf[:, 0:2048], rowbuf[:, 2048:4096], op=MAX)
            t2 = stg.tile([128, 1024], f16, tag="t2", name="t2")
            nc.vector.tensor_tensor(t2[:], t1[:, 0:1024], t1[:, 1024:2048], op=MAX)
            t3 = stg.tile([128, 512], f16, tag="t3", name="t3")
            nc.vector.tensor_tensor(t3[:], t2[:, 0:512], t2[:, 512:1024], op=MAX)
            t4 = stg.tile([128, 256], f16, tag="t4", name="t4")
            nc.vector.tensor_tensor(t4[:], t3[:, 0:256], t3[:, 256:512], op=MAX)
            nc.vector.tensor_reduce(rowmax[:, pt:pt + 1], t4[:],
                                    axis=mybir.AxisListType.X, op=MAX)

        # ================= tails =================
        nc.sync.dma_start(o_row.ap(), rowmax[:])
        for g in range(NG):
            colred = work.tile([128, FC], f32, tag=f"colred{g}", name=f"colred{g}")
            nc.gpsimd.partition_all_reduce(
                colred[:], colacc[g][:], 128, bass_isa.ReduceOp.max)
            nc.sync.dma_start(o_col.ap()[g * FC:(g + 1) * FC], colred[0:1, :])

    nc.compile()
    return nc


def _get_nc():
    if "nc" not in _cache:
        _cache["nc"] = _build_program()
    return _cache["nc"]


def _register_ntff_hook():
    """This image's antenv lacks axon_hooks; register the NTFF profile hook
    ourselves so run_bass_kernel_spmd(trace=True) can neuron-profile."""
    import sys, types
    if "antenv.axon_hooks" in sys.modules:
        return
    try:
        from trn_agent_boot.trn_boot import _ntff_profile_via_ctypes
        hook = _ntff_profile_via_ctypes("/opt/axon/libaxon_pjrt.so")
        mod = types.ModuleType("antenv.axon_hooks")
        mod.get_axon_ntff_profile_hook = lambda: hook
        mod.set_axon_ntff_profile_hook = lambda h: None
        sys.modules["antenv.axon_hooks"] = mod
        from concourse import bass_utils
        bass_utils.upload_artifacts = lambda tmpdir: tmpdir
    except Exception:
        pass


def _run(in_maps, trace=False):
    from concourse.bass_utils import run_bass_kernel_spmd
    if trace:
        _register_ntff_hook()
    nc = _get_nc()
    return run_bass_kernel_spmd(nc, in_maps, list(range(NCORES)), trace=trace)


def _combine(results):
    minx_sum = 0.0
    miny_sum = 0.0
    kl_sum = 0.0
    for r in results:
        minx_sum += -(r["o_row"].astype(np.float64).sum())
        miny_sum += -(r["o_col"].astype(np.float64).sum())
        kl_sum += r["o_kl"].astype(np.float64).sum()
    recon = minx_sum / (NCORES * N) + miny_sum / (NCORES * N)
    kld = -0.5 * (B * LATENT * 1.0 + kl_sum) / B
    total = recon + BETA * kld
    return (np.float32(total), np.float32(recon), np.float32(kld))


def kernel(recon_x, x, mu, logvar, _trace=False):
    recon_x = np.ascontiguousarray(recon_x, dtype=np.float32)
    x = np.ascontiguousarray(x, dtype=np.float32)
    mu = np.ascontiguousarray(mu, dtype=np.float32)
    logvar = np.ascontiguousarray(logvar, dtype=np.float32)
    in_maps = [
        {"xr": recon_x[c], "xx": x[c], "mu": mu[c], "lv": logvar[c]}
        for c in range(NCORES)
    ]
    res = _run(in_maps, trace=_trace)
    out = _combine(res.results)
    if _trace:
        return out, res
    return out
